# revision 11
# baseline (speedup 1.0000x reference)
"""Trainium2 Bass kernel for nn_Compute_all_u (embedding gather + batched affine dot).

For each voxel v:
    u[v, :] = C[e_v, 0, :] + x_v*C[e_v, 1, :] + y_v*C[e_v, 2, :] + z_v*C[e_v, 3, :]
where e_v = voxels_elements[v], (x,y,z) = all_voxels_centroids[v].

Strategy ("broadcast-R"): shard the ELEMENT TABLE across the 8 cores
(62,500 elements each) and route voxels to the core owning their element.
Each element is then referenced ~16x per core (Poisson(16)), so the device
never needs data-dependent addressing: the host sorts voxels by element and
packs each element's voxels into groups of consecutive slots that share one
(host-repeated) table row; the device streams rows + slot-ordered centroids
and broadcasts each row across its group with stride-0 access patterns.

This removes the SWDGE dma_gather entirely - the v1 kernel was bottlenecked
at ~8.7ns/row of Q7 descriptor generation (1M rows / 4 queues = 2.26ms),
with DMA engines only ~14% busy. Here everything is sequential DMA + DVE.
(Offloading a slice to the Pool engine was tried and REGRESSED: co-running
Pool with DVE halves both engines' SBUF throughput - kept all-DVE.)

MIXED GROUP SIZES cut slot padding: an element with count L gets
floor(L/8) full R=8 groups, plus (if the remainder m=L%8 is 5..7) one more
R=8 group, while small remainders m=1..4 go to a separate R=4 region.
Seed-0 slots: 1.11M vs 1.23M for uniform R=8 (~10% less DVE+DMA work).

Layouts are PLANAR so every DVE operand has innermost stride 1 (the 2x_1P
fp16 perf mode requires step_x=+-1 / 4B alignment on all srcs and dst;
broadcasts live on outer axes where stride 0 is allowed):
  trow[t, p, dk, c]   dk = d*3+k       (12 planes of cg rows)
  cent[t, p, j, r, c] j in {x,y,z}     (3 planes of Rreg x cg)
with group g mapped tile-major / partition / column, slots s = g*Rreg + r.

Per tile the 6 fp16 DVE ops (out shape [128, 3, Rreg, cg]) are:
  tmp = X(bcast k) * C1(bcast r);  u  = C0(bcast r) + tmp
  tmp = Y(bcast k) * C2(bcast r);  u += tmp
  tmp = Z(bcast k) * C3(bcast r);  u += tmp

Tiles are SIZE-GRADED (small head tiles) so the first DVE op only waits on
a quarter-size DMA; output stores issue from the Activation engine's HWDGE
queue so tile loads never queue behind them.

Precision: fp16 throughout; measured rel err ~1e-3 vs the f32 reference
(gate 2e-2): values are O(1) normals, u ~ N(0, 4), fp16 eps 9.8e-4.

Host prep per call: one 8M argsort by element, per-core bincount/cumsum to
assign slots, np.repeat to build the group row streams, scatter centroids
into slot-planar order, un-permute outputs. Any voxel whose slot would
exceed a region capacity (seed-0 actual: A 121,418/122,880; B 31,153/32,768)
falls back to exact host math.
"""

import numpy as np

from concourse import bacc, bass, tile, mybir
from concourse.bass_utils import run_bass_kernel_spmd

N_VOXELS = 8_000_000
N_ELEM = 500_000
N_CORES = 8
EPC = N_ELEM // N_CORES     # 62,500 elements per core
RA = 8                      # region-A slots per group
RB = 4                      # region-B slots per group (small remainders)

# device tile schedule: (region, n_tiles, group-columns per partition, R)
# A capacity 960 cols = 122,880 groups; B capacity 256 cols = 32,768 groups
TILES = (
    ("A", 4, 30, RA),       # small head tiles: compute starts early
    ("A", 6, 120, RA),
    ("B", 2, 128, RB),
    ("A", 2, 60, RA),       # small tail tiles: quick drain
)
CAP_A = sum(n * 128 * cg for rg, n, cg, _ in TILES if rg == "A")   # 122,880
CAP_B = sum(n * 128 * cg for rg, n, cg, _ in TILES if rg == "B")   # 32,768
NSLOT_A = CAP_A * RA        # 983,040
NSLOT_B = CAP_B * RB        # 131,072
NSLOT = NSLOT_A + NSLOT_B   # 1,114,112 slots per core

f16 = mybir.dt.float16


def build_nc() -> bass.Bass:
    nc = bacc.Bacc("TRN2")
    params = []
    for i, (rg, n, cg, r) in enumerate(TILES):
        params.append((
            # trow planes (12*cg) and cent planes (3*r*cg) packed in one
            # param so each tile costs a single load on the Sync queue
            nc.declare_dram_parameter(
                f"tc{i}", [n, 128, (12 + 3 * r) * cg], f16, isOutput=False
            ),
            nc.declare_dram_parameter(f"out{i}", [n, 128, 3 * r * cg], f16, isOutput=True),
        ))

    mul = mybir.AluOpType.mult
    add = mybir.AluOpType.add

    # per-class pools: the pool allocates bufs slots per distinct tag, so one
    # global deep pool over 4 tile-size classes overflows SBUF
    class_bufs = [min(n + 1, 5) for _, n, _, _ in TILES]

    with tile.TileContext(nc) as tc:
        with (
            tc.tile_pool(name="io0", bufs=class_bufs[0]) as p0,
            tc.tile_pool(name="io1", bufs=class_bufs[1]) as p1,
            tc.tile_pool(name="io2", bufs=class_bufs[2]) as p2,
            tc.tile_pool(name="io3", bufs=class_bufs[3]) as p3,
            tc.tile_pool(name="tmp", bufs=2) as tmp_pool,
        ):
            pools = [p0, p1, p2, p3]
            for i, (rg, n, cg, r) in enumerate(TILES):
                tc_in, out = params[i]
                io_pool = pools[i]
                for t in range(n):
                    tc_t = io_pool.tile([128, (12 + 3 * r) * cg], f16, tag=f"tc{i}")
                    nc.sync.dma_start(out=tc_t[:], in_=tc_in[t])

                    u = io_pool.tile([128, 3 * r * cg], f16, tag=f"u{i}")
                    tmp = tmp_pool.tile([128, 3 * r * cg], f16, tag=f"t{i}")

                    tr = tc_t[:, 0:12 * cg].rearrange("p (dk c) -> p dk c", c=cg)
                    cr = tc_t[:, 12 * cg:].rearrange("p (j r c) -> p j r c", r=r, c=cg)
                    ur = u[:].rearrange("p (k r c) -> p k r c", r=r, c=cg)
                    tmr = tmp[:].rearrange("p (k r c) -> p k r c", r=r, c=cg)

                    def rows(d):  # trow planes d*3..d*3+3, bcast over r
                        return tr[:, 3 * d:3 * d + 3, :].unsqueeze(2).to_broadcast(
                            [128, 3, r, cg]
                        )

                    def xyz(j):  # cent plane j, bcast over k
                        return cr[:, j:j + 1, :, :].to_broadcast([128, 3, r, cg])

                    nc.vector.tensor_tensor(out=tmr, in0=xyz(0), in1=rows(1), op=mul)
                    nc.vector.tensor_tensor(out=ur, in0=rows(0), in1=tmr, op=add)
                    nc.vector.tensor_tensor(out=tmr, in0=xyz(1), in1=rows(2), op=mul)
                    nc.vector.tensor_tensor(out=ur, in0=ur, in1=tmr, op=add)
                    nc.vector.tensor_tensor(out=tmr, in0=xyz(2), in1=rows(3), op=mul)
                    nc.vector.tensor_tensor(out=ur, in0=ur, in1=tmr, op=add)

                    # stores ride the Activation engine's HWDGE queue so the
                    # next tiles' loads (Sync queue) are never stuck behind them
                    nc.scalar.dma_start(out=out[t], in_=u[:])
    nc.finalize()
    return nc


_NC_CACHE: dict = {}


def _get_nc():
    if TILES not in _NC_CACHE:
        _NC_CACHE[TILES] = build_nc()
    return _NC_CACHE[TILES]


def _prep_core(el, vox, coeffs16_c, cent16_full):
    """Build one core's device arrays from its (sorted) local element ids."""
    n = el.shape[0]
    L = np.bincount(el, minlength=EPC)
    q, m = L // RA, L % RA
    a_grp = q + (m >= 5)                     # R=8 groups per element
    b_grp = ((m >= 1) & (m <= 4)).astype(np.int64)   # 0/1 R=4 groups

    a_base = np.zeros(EPC, dtype=np.int64)
    np.cumsum(a_grp[:-1], out=a_base[1:])
    b_base = np.zeros(EPC, dtype=np.int64)
    np.cumsum(b_grp[:-1], out=b_base[1:])
    run_start = np.zeros(EPC, dtype=np.int64)
    np.cumsum(L[:-1], out=run_start[1:])

    rank = np.arange(n, dtype=np.int64) - run_start[el]
    athr = a_grp[el] * RA                    # slots this element owns in A
    in_a = rank < athr
    slot = np.where(
        in_a,
        a_base[el] * RA + rank,
        NSLOT_A + b_base[el] * RB + (rank - athr),
    )
    ok = np.where(in_a, slot < NSLOT_A, slot < NSLOT)

    trow_a = np.zeros((CAP_A, 12), dtype=np.float16)
    tot_a = int(a_grp.sum())
    rep = np.repeat(coeffs16_c, a_grp, axis=0)
    trow_a[:min(tot_a, CAP_A)] = rep[:CAP_A]
    trow_b = np.zeros((CAP_B, 12), dtype=np.float16)
    sel_b = coeffs16_c[b_grp.astype(bool)]
    trow_b[:min(sel_b.shape[0], CAP_B)] = sel_b[:CAP_B]

    cent_slot = np.zeros((NSLOT, 3), dtype=np.float16)
    cent_slot[slot[ok]] = cent16_full[vox[ok]]

    # slice group-major streams into per-tile-region planar arrays
    in_map = {}
    gA = gB = 0
    for i, (rg, nt, cg, r) in enumerate(TILES):
        ng = nt * 128 * cg
        if rg == "A":
            rows = trow_a[gA:gA + ng]
            cent = cent_slot[gA * RA:(gA + ng) * RA]
            gA += ng
        else:
            rows = trow_b[gB:gB + ng]
            cent = cent_slot[NSLOT_A + gB * RB:NSLOT_A + (gB + ng) * RB]
            gB += ng
        trow_p = rows.reshape(nt, 128, cg, 12).transpose(0, 1, 3, 2).reshape(
            nt, 128, 12 * cg
        )
        cent_p = cent.reshape(nt, 128, cg, r, 3).transpose(0, 1, 4, 3, 2).reshape(
            nt, 128, 3 * r * cg
        )
        in_map[f"tc{i}"] = np.ascontiguousarray(
            np.concatenate([trow_p, cent_p], axis=2)
        )

    return in_map, slot, ok


def _reassemble(results_c):
    """Concatenate per-tile-region outputs back to [NSLOT, 3] in slot order."""
    parts_a, parts_b = [], []
    for i, (rg, nt, cg, r) in enumerate(TILES):
        blk = results_c[f"out{i}"].reshape(nt, 128, 3, r, cg)
        flat = np.ascontiguousarray(blk.transpose(0, 1, 4, 3, 2)).reshape(-1, 3)
        (parts_a if rg == "A" else parts_b).append(flat)
    return np.concatenate(parts_a + parts_b, axis=0)


def kernel(all_coeffs, all_voxels_centroids, voxels_elements, _trace=False, **run_kwargs):
    nc = _get_nc()
    coeffs12 = np.asarray(all_coeffs, dtype=np.float32).reshape(N_ELEM, 12)
    coeffs16 = coeffs12.astype(np.float16)
    cent_full = np.asarray(all_voxels_centroids, dtype=np.float32)
    cent16 = cent_full.astype(np.float16)
    e_full = np.asarray(voxels_elements).astype(np.int64)

    order = np.argsort(e_full, kind="stable")
    es = e_full[order]
    bounds = np.searchsorted(es, np.arange(N_CORES + 1, dtype=np.int64) * EPC)

    in_maps, metas = [], []
    for c in range(N_CORES):
        lo, hi = int(bounds[c]), int(bounds[c + 1])
        vox = order[lo:hi]
        el = (es[lo:hi] - c * EPC).astype(np.int64)
        m, slot, ok = _prep_core(el, vox, coeffs16[c * EPC:(c + 1) * EPC], cent16)
        in_maps.append(m)
        metas.append((vox, slot, ok))

    res = run_bass_kernel_spmd(
        nc, in_maps, core_ids=list(range(N_CORES)), trace=_trace, **run_kwargs
    )

    full = np.empty((N_VOXELS, 3), dtype=np.float32)
    for c in range(N_CORES):
        vox, slot, ok = metas[c]
        u_slots = _reassemble(res.results[c])
        full[vox[ok]] = u_slots[slot[ok]].astype(np.float32)
        bad = ~ok
        if bad.any():
            vb = vox[bad]
            cf = coeffs12[e_full[vb]].reshape(-1, 4, 3)
            xyz = cent_full[vb]
            full[vb] = cf[:, 0] + np.einsum("nd,ndk->nk", xyz, cf[:, 1:4])
    if _trace:
        return full, res
    return full


# revision 12
# speedup vs baseline: 1.0462x; 1.0462x over previous
"""Trainium2 Bass kernel for nn_Compute_all_u (embedding gather + batched affine dot).

For each voxel v:
    u[v, :] = C[e_v, 0, :] + x_v*C[e_v, 1, :] + y_v*C[e_v, 2, :] + z_v*C[e_v, 3, :]
where e_v = voxels_elements[v], (x,y,z) = all_voxels_centroids[v].

Strategy ("broadcast-R"): shard the ELEMENT TABLE across the 8 cores
(62,500 elements each) and route voxels to the core owning their element.
Each element is then referenced ~16x per core (Poisson(16)), so the device
never needs data-dependent addressing: the host sorts voxels by element and
packs each element's voxels into groups of consecutive slots that share one
(host-repeated) table row; the device streams rows + slot-ordered centroids
and broadcasts each row across its group with stride-0 access patterns.

This removes the SWDGE dma_gather entirely - the v1 kernel was bottlenecked
at ~8.7ns/row of Q7 descriptor generation (1M rows / 4 queues = 2.26ms),
with DMA engines only ~14% busy. Here everything is sequential DMA + DVE.
(Offloading a slice to the Pool engine was tried and REGRESSED: co-running
Pool with DVE halves both engines' SBUF throughput - kept all-DVE.)

MIXED GROUP SIZES cut slot padding: an element with count L gets
floor(L/8) full R=8 groups in region A (plus one more if the remainder
m=L%8 is 5..7), while remainders m=3..4 go to an R=4 region B and m=1..2
to an R=2 region C. Seed-0 slots: 1.08M vs 1.23M for uniform R=8.

Layouts are PLANAR so every DVE operand has innermost stride 1 (the 2x_1P
fp16 perf mode requires step_x=+-1 / 4B alignment on all srcs and dst;
broadcasts live on outer axes where stride 0 is allowed). Each tile's rows
and centroids are packed into ONE dram param (single load per tile):
  tc[t, p, 0:12*cg]        trow planes, dk = d*3+k
  tc[t, p, 12*cg:]         cent planes [j, r, c], j in {x,y,z}
with group g mapped tile-major / partition / column, slots s = g*Rreg + r.

Per tile the 6 fp16 DVE ops (out shape [128, 3, Rreg, cg]) are:
  tmp = X(bcast k) * C1(bcast r);  u  = C0(bcast r) + tmp
  tmp = Y(bcast k) * C2(bcast r);  u += tmp
  tmp = Z(bcast k) * C3(bcast r);  u += tmp

Tiles are SIZE-GRADED (8->24->88 column head ramp, 4x180 mids, 60-column
tail) so the first DVE op waits only on a ~70KB load and the drain is
short; output stores issue from the Activation engine's HWDGE queue so
tile loads (Sync queue) never wait behind them.

Precision: fp16 throughout; measured rel err ~1e-3 vs the f32 reference
(gate 2e-2): values are O(1) normals, u ~ N(0, 4), fp16 eps 9.8e-4.

Host prep per call: one 8M argsort by element, per-core bincount/cumsum to
assign slots, np.repeat to build the group row streams, scatter centroids
into slot-planar order, un-permute outputs. Any voxel whose slot would
exceed a region capacity (seed-0 actual: A 121,418/122,880; B 15,589/16,384;
C 15,601/16,384) falls back to exact host math.
"""

import numpy as np

from concourse import bacc, bass, tile, mybir
from concourse.bass_utils import run_bass_kernel_spmd

N_VOXELS = 8_000_000
N_ELEM = 500_000
N_CORES = 8
EPC = N_ELEM // N_CORES     # 62,500 elements per core
RA, RB, RC = 8, 4, 2

# device tile schedule: (region, n_tiles, group-columns per partition, R, bufs)
TILES = (
    ("A", 1, 8, RA, 2),     # micro head: compute starts ~0.3us after barrier
    ("A", 1, 24, RA, 2),
    ("A", 1, 88, RA, 2),
    ("A", 4, 180, RA, 3),   # big mids: fewer per-op overheads
    ("B", 1, 128, RB, 2),
    ("C", 1, 128, RC, 2),
    ("A", 2, 60, RA, 2),    # small tail: quick drain
)
CAP = {r: sum(n * 128 * cg for rg, n, cg, _, _ in TILES if rg == r)
       for r in ("A", "B", "C")}          # A: 122,880  B: 16,384  C: 16,384
NSLOT_A = CAP["A"] * RA                   # 983,040
NSLOT_B = CAP["B"] * RB                   # 65,536
NSLOT_C = CAP["C"] * RC                   # 32,768
NSLOT = NSLOT_A + NSLOT_B + NSLOT_C       # 1,081,344 slots per core

f16 = mybir.dt.float16


def build_nc() -> bass.Bass:
    nc = bacc.Bacc("TRN2")
    params = []
    for i, (rg, n, cg, r, _) in enumerate(TILES):
        params.append((
            nc.declare_dram_parameter(
                f"tc{i}", [n, 128, (12 + 3 * r) * cg], f16, isOutput=False
            ),
            nc.declare_dram_parameter(f"out{i}", [n, 128, 3 * r * cg], f16, isOutput=True),
        ))

    mul = mybir.AluOpType.mult
    add = mybir.AluOpType.add

    with tile.TileContext(nc) as tc:
        import contextlib
        with contextlib.ExitStack() as stack:
            pools = [
                stack.enter_context(tc.tile_pool(name=f"io{i}", bufs=b))
                for i, (_, _, _, _, b) in enumerate(TILES)
            ]
            tmp_pool = stack.enter_context(tc.tile_pool(name="tmp", bufs=2))

            for i, (rg, n, cg, r, _) in enumerate(TILES):
                tc_in, out = params[i]
                io_pool = pools[i]
                for t in range(n):
                    tc_t = io_pool.tile([128, (12 + 3 * r) * cg], f16, tag=f"tc{i}")
                    nc.sync.dma_start(out=tc_t[:], in_=tc_in[t])

                    u = io_pool.tile([128, 3 * r * cg], f16, tag=f"u{i}")
                    tmp = tmp_pool.tile([128, 3 * r * cg], f16, tag=f"t{i}")

                    tr = tc_t[:, 0:12 * cg].rearrange("p (dk c) -> p dk c", c=cg)
                    cr = tc_t[:, 12 * cg:].rearrange("p (j r c) -> p j r c", r=r, c=cg)
                    ur = u[:].rearrange("p (k r c) -> p k r c", r=r, c=cg)
                    tmr = tmp[:].rearrange("p (k r c) -> p k r c", r=r, c=cg)

                    def rows(d):  # trow planes d*3..d*3+3, bcast over r
                        return tr[:, 3 * d:3 * d + 3, :].unsqueeze(2).to_broadcast(
                            [128, 3, r, cg]
                        )

                    def xyz(j):  # cent plane j, bcast over k
                        return cr[:, j:j + 1, :, :].to_broadcast([128, 3, r, cg])

                    nc.vector.tensor_tensor(out=tmr, in0=xyz(0), in1=rows(1), op=mul)
                    nc.vector.tensor_tensor(out=ur, in0=rows(0), in1=tmr, op=add)
                    nc.vector.tensor_tensor(out=tmr, in0=xyz(1), in1=rows(2), op=mul)
                    nc.vector.tensor_tensor(out=ur, in0=ur, in1=tmr, op=add)
                    nc.vector.tensor_tensor(out=tmr, in0=xyz(2), in1=rows(3), op=mul)
                    nc.vector.tensor_tensor(out=ur, in0=ur, in1=tmr, op=add)

                    # stores ride the Activation engine's HWDGE queue so the
                    # next tiles' loads (Sync queue) never wait behind them
                    nc.scalar.dma_start(out=out[t], in_=u[:])
    nc.finalize()
    return nc


_NC_CACHE: dict = {}


def _get_nc():
    if TILES not in _NC_CACHE:
        _NC_CACHE[TILES] = build_nc()
    return _NC_CACHE[TILES]


def _prep_core(el, vox, coeffs16_c, cent16_full):
    """Build one core's device arrays from its (sorted) local element ids."""
    n = el.shape[0]
    L = np.bincount(el, minlength=EPC)
    q, m = L // RA, L % RA
    a_grp = q + (m >= 5)                             # R=8 groups per element
    b_grp = ((m >= 3) & (m <= 4)).astype(np.int64)   # 0/1 R=4 groups
    c_grp = ((m >= 1) & (m <= 2)).astype(np.int64)   # 0/1 R=2 groups

    a_base = np.zeros(EPC, dtype=np.int64)
    np.cumsum(a_grp[:-1], out=a_base[1:])
    b_base = np.zeros(EPC, dtype=np.int64)
    np.cumsum(b_grp[:-1], out=b_base[1:])
    c_base = np.zeros(EPC, dtype=np.int64)
    np.cumsum(c_grp[:-1], out=c_base[1:])
    run_start = np.zeros(EPC, dtype=np.int64)
    np.cumsum(L[:-1], out=run_start[1:])

    rank = np.arange(n, dtype=np.int64) - run_start[el]
    athr = a_grp[el] * RA                    # slots this element owns in A
    in_a = rank < athr
    in_b = b_grp[el].astype(bool)
    rem = rank - athr
    slot = np.where(
        in_a,
        a_base[el] * RA + rank,
        np.where(
            in_b,
            NSLOT_A + b_base[el] * RB + rem,
            NSLOT_A + NSLOT_B + c_base[el] * RC + rem,
        ),
    )
    ok = np.where(
        in_a,
        slot < NSLOT_A,
        np.where(in_b, slot < NSLOT_A + NSLOT_B, slot < NSLOT),
    )

    def _rows_for(grp, cap, repeat):
        buf = np.zeros((cap, 12), dtype=np.float16)
        if repeat:
            rep = np.repeat(coeffs16_c, grp, axis=0)
        else:
            rep = coeffs16_c[grp.astype(bool)]
        buf[:min(rep.shape[0], cap)] = rep[:cap]
        return buf

    trow = {
        "A": _rows_for(a_grp, CAP["A"], True),
        "B": _rows_for(b_grp, CAP["B"], False),
        "C": _rows_for(c_grp, CAP["C"], False),
    }

    cent_slot = np.zeros((NSLOT, 3), dtype=np.float16)
    cent_slot[slot[ok]] = cent16_full[vox[ok]]

    # slice group-major streams into per-tile-class planar arrays
    reg_R = {"A": RA, "B": RB, "C": RC}
    reg_slot0 = {"A": 0, "B": NSLOT_A, "C": NSLOT_A + NSLOT_B}
    gpos = {"A": 0, "B": 0, "C": 0}
    in_map = {}
    for i, (rg, nt, cg, r, _) in enumerate(TILES):
        ng = nt * 128 * cg
        g0 = gpos[rg]
        rows = trow[rg][g0:g0 + ng]
        s0 = reg_slot0[rg] + g0 * r
        cent = cent_slot[s0:s0 + ng * r]
        gpos[rg] = g0 + ng
        trow_p = rows.reshape(nt, 128, cg, 12).transpose(0, 1, 3, 2).reshape(
            nt, 128, 12 * cg
        )
        cent_p = cent.reshape(nt, 128, cg, r, 3).transpose(0, 1, 4, 3, 2).reshape(
            nt, 128, 3 * r * cg
        )
        in_map[f"tc{i}"] = np.ascontiguousarray(
            np.concatenate([trow_p, cent_p], axis=2)
        )

    return in_map, slot, ok


def _reassemble(results_c):
    """Concatenate per-tile outputs back to [NSLOT, 3] in slot order."""
    parts = {"A": [], "B": [], "C": []}
    for i, (rg, nt, cg, r, _) in enumerate(TILES):
        blk = results_c[f"out{i}"].reshape(nt, 128, 3, r, cg)
        parts[rg].append(
            np.ascontiguousarray(blk.transpose(0, 1, 4, 3, 2)).reshape(-1, 3)
        )
    return np.concatenate(parts["A"] + parts["B"] + parts["C"], axis=0)


def kernel(all_coeffs, all_voxels_centroids, voxels_elements, _trace=False, **run_kwargs):
    nc = _get_nc()
    coeffs12 = np.asarray(all_coeffs, dtype=np.float32).reshape(N_ELEM, 12)
    coeffs16 = coeffs12.astype(np.float16)
    cent_full = np.asarray(all_voxels_centroids, dtype=np.float32)
    cent16 = cent_full.astype(np.float16)
    e_full = np.asarray(voxels_elements).astype(np.int64)

    order = np.argsort(e_full, kind="stable")
    es = e_full[order]
    bounds = np.searchsorted(es, np.arange(N_CORES + 1, dtype=np.int64) * EPC)

    in_maps, metas = [], []
    for c in range(N_CORES):
        lo, hi = int(bounds[c]), int(bounds[c + 1])
        vox = order[lo:hi]
        el = (es[lo:hi] - c * EPC).astype(np.int64)
        m, slot, ok = _prep_core(el, vox, coeffs16[c * EPC:(c + 1) * EPC], cent16)
        in_maps.append(m)
        metas.append((vox, slot, ok))

    res = run_bass_kernel_spmd(
        nc, in_maps, core_ids=list(range(N_CORES)), trace=_trace, **run_kwargs
    )

    full = np.empty((N_VOXELS, 3), dtype=np.float32)
    for c in range(N_CORES):
        vox, slot, ok = metas[c]
        u_slots = _reassemble(res.results[c])
        full[vox[ok]] = u_slots[slot[ok]].astype(np.float32)
        bad = ~ok
        if bad.any():
            vb = vox[bad]
            cf = coeffs12[e_full[vb]].reshape(-1, 4, 3)
            xyz = cent_full[vb]
            full[vb] = cf[:, 0] + np.einsum("nd,ndk->nk", xyz, cf[:, 1:4])
    if _trace:
        return full, res
    return full
